# revision 1
# baseline (speedup 1.0000x reference)
"""BiPairwiseNegativeCELoss Trainium2 kernel (8-core data-parallel).

loss = ( mean(softplus(neg - pos)) + mean(softplus(neg_ib - pos)) ) / 2
  pos    = rowwise dot(q, d)                (diag of q @ d.T)
  neg    = rowwise dot(q, nd)
  neg_ib = rowmax of (q @ d.T - BIG*eye)    (hardest in-batch negative)

Sharding: batch rows split across 8 cores (2048 rows each); every core
streams the full doc matrix as the matmul moving operand.

The row-max over 16384 columns is the bottleneck (PSUM can only be read
by the Vector/Scalar engines at ~1 elem/lane/cycle). We halve it with a
pair-max decomposition:

  max(a, b) = (a+b)/2 + |(a-b)/2|

  ssum = q @ DsumT  (Dsum = (d_even + d_odd)/2)      [TensorE]
  sdif = q @ DdifT  (Ddif = (d_even - d_odd)/2)      [TensorE]
  |sdif|           PSUM -> SBUF                       [ScalarE Abs]
  rowmax(ssum + |sdif|), seeded/chained per chunk     [VectorE, one
      fused custom-DVE op: body=Src0+Src1, accum=maxx, seed=C1]

The fused op is registered at import time into concourse's custom-DVE
table mechanism (uop table ships inside the NEFF).

The diagonal pair {s_ii, s_i,i^1} is excluded with a -1e6 "half-eye"
mask on the ssum bank; the partner score s_i,i^1 is re-added exactly as
the chunk-0 seed (computed as a rowwise dot). Per-core pair-columns are
rotated so every core's diagonal block lands in chunk 0 at the same
static position (keeps the program SPMD).
Softplus + means run on the host in float64 on the tiny per-row vectors.
"""

import numpy as np
import ml_dtypes

import concourse.bacc as bacc
import concourse.tile as tile
import concourse.mybir as mybir
import concourse.dve_ops as dve_ops
from concourse.dve_spec import Spec, Src0, Src1, C1, maxx, lower, _has_src1
from concourse.dve_uop import DveOpSpec
from concourse.bass_utils import run_bass_kernel_spmd
from contextlib import ExitStack

B = 16384          # batch
D = 128            # embedding dim
NCORES = 8
R = B // NCORES    # rows per core = 2048
M_TILES = R // 128          # 16 row tiles per core
PC = B // 2                 # pair columns = 8192
CHUNK = 1024                # pair columns per pipeline iteration
N_CHUNKS = PC // CHUNK      # 8
MM_N = 512                  # moving free dim per matmul
BIG = 1e6

_COMPILED = None


def _ref_tt_add_maxred(in0, in1, c0, c1, c2):
    P = in0.shape[0]
    body = (in0.astype(np.float32).reshape(P, -1)
            + np.asarray(in1, np.float32).reshape(P, -1))
    return body, dve_ops._accum_ref(body, c1, maxx, False)


def _register_fused_op():
    """out = in0 + in1 ; accum_out = max(rowmax(out), seed[C1])."""
    name = "TT_ADD_MAXREDUCE_ANT"
    if name in dve_ops._SUB_OPCODE_FOR_NAME:
        return next(op for op in dve_ops.OPS if op.name == name)
    op = dve_ops.DveOp(
        name,
        Spec(body=Src0 + Src1, accum=maxx, accum_init=C1,
             reference=_ref_tt_add_maxred),
        subdim=False,
        uops_sha={},
    )
    row = max(dve_ops._SUB_OPCODE_FOR_NAME.values()) + 1
    assert row < 0x20
    dve_ops.OPS.append(op)
    dve_ops.CUSTOM_DVE_SPECS[name] = op.spec
    dve_ops._SUB_OPCODE_FOR_NAME[name] = row
    for ver in ("v3", "v4"):
        spec = DveOpSpec(name=name, opcode=row, uops=lower(op.spec, ver=ver),
                         rd1_en=_has_src1(op.spec))
        op.uops_sha[ver] = spec.sha(ver)
    return op


FUSED_OP = _register_fused_op()


def _build(repeat=1, absd_f16=True, prefetch_dif=False, no_act=False, no_dve=False):
    fp32, bf16 = mybir.dt.float32, mybir.dt.bfloat16
    absd_dt = mybir.dt.float16 if absd_f16 else fp32
    nc = bacc.Bacc("TRN2", target_bir_lowering=False, debug=False)

    qT_d = nc.dram_tensor("qT", [D, R], bf16, kind="ExternalInput")
    dsumT_d = nc.dram_tensor("dsumT", [D, PC], bf16, kind="ExternalInput")
    ddifT_d = nc.dram_tensor("ddifT", [D, PC], bf16, kind="ExternalInput")
    # (q±x) shards for the rowwise dots via the square trick:
    # 4*q.x = sum((q+x)^2) - sum((q-x)^2), accumulated on ScalarE
    dot_names = ["qd_s", "qd_d", "qn_s", "qn_d", "qw_s", "qw_d"]
    dot_drams = {n: nc.dram_tensor(n, [R, D], fp32, kind="ExternalInput")
                 for n in dot_names}
    heye_d = nc.dram_tensor("heye", [D, 64], fp32, kind="ExternalInput")
    out_d = nc.dram_tensor("out", [5, D, M_TILES], fp32, kind="ExternalOutput")

    with tile.TileContext(nc) as tc, ExitStack() as ctx:
        resid = ctx.enter_context(tc.tile_pool(name="resid", bufs=1))
        dots_in = ctx.enter_context(tc.tile_pool(name="dots_in", bufs=3))
        absp = ctx.enter_context(tc.tile_pool(name="absp", bufs=4))
        small = ctx.enter_context(tc.tile_pool(name="small", bufs=1))
        trashp = ctx.enter_context(tc.tile_pool(name="trashp", bufs=2))
        psum_dif = ctx.enter_context(tc.tile_pool(name="psum_dif", bufs=2, space="PSUM"))
        psum_sum = ctx.enter_context(tc.tile_pool(name="psum_sum", bufs=2, space="PSUM"))

        # resident operands
        qT = resid.tile([D, R], bf16, name="qT_t")
        dsumT = resid.tile([D, PC], bf16, name="dsumT_t")
        ddifT = resid.tile([D, PC], bf16, name="ddifT_t")
        heye = resid.tile([D, 64], fp32, name="heye_t")

        nc.sync.dma_start(qT[:], qT_d.ap())
        nc.sync.dma_start(heye[:], heye_d.ap())
        for ci in range(N_CHUNKS):
            sl = slice(ci * CHUNK, (ci + 1) * CHUNK)
            nc.sync.dma_start(ddifT[:, sl], ddifT_d.ap()[:, sl])
            nc.sync.dma_start(dsumT[:, sl], dsumT_d.ap()[:, sl])

        # staging for per-row results
        accs = {n: small.tile([D, M_TILES], fp32, name=f"acc_{n}")
                for n in dot_names}
        par_acc = small.tile([D, M_TILES], fp32, name="par_acc")
        # chain[ci] holds the running rowmax after chunk ci (per m-tile col)
        chain = [small.tile([D, M_TILES], fp32, name=f"chain_{ci}")
                 for ci in range(N_CHUNKS)]

        # ---- rowwise dots via ScalarE Square+accumulate, early ----
        f16 = mybir.dt.float16
        for m in range(M_TILES):
            rs = slice(m * 128, (m + 1) * 128)
            for n in dot_names:
                xt = dots_in.tile([128, D], fp32, name=f"dot_{n}")
                nc.sync.dma_start(xt[:], dot_drams[n].ap()[rs, :])
                tr = trashp.tile([128, D], f16, name="dot_trash")
                nc.scalar.activation(tr[:], xt[:],
                                     mybir.ActivationFunctionType.Square,
                                     accum_out=accs[n][:, m : m + 1])
        # partner seed: host pre-scales (q±w) by 1/2, so the squared-sum
        # difference is exactly q.w
        nc.vector.tensor_tensor(par_acc[:], accs["qw_s"][:], accs["qw_d"][:],
                                op=mybir.AluOpType.subtract)

        # ---- pair-max pipeline ----
        loop_cm = ExitStack()
        if repeat > 1:
            loop_cm.enter_context(tc.For_i(
                0, repeat, 1,
                hint_engines=(mybir.EngineType.PE, mybir.EngineType.DVE,
                              mybir.EngineType.Activation)))
        iters = [(ci, m) for ci in range(N_CHUNKS) for m in range(M_TILES)]

        def emit_dif(ci, m):
            dif = psum_dif.tile([128, CHUNK], fp32, name="dif_bank")
            w = qT[:, m * 128 : (m + 1) * 128]
            for h in range(CHUNK // MM_N):
                cs = slice(ci * CHUNK + h * MM_N, ci * CHUNK + (h + 1) * MM_N)
                nc.tensor.matmul(dif[:, h * MM_N : (h + 1) * MM_N], w,
                                 ddifT[:, cs], start=True, stop=True)
            return dif

        absd_static = resid.tile([128, CHUNK], absd_dt, name="absd_static")
        if no_act:
            nc.vector.memset(absd_static[:], 0.25)
        if no_dve:
            for ci in range(N_CHUNKS):
                nc.vector.memset(chain[ci][:], 0.0)

        difs = {}
        if prefetch_dif:
            difs[iters[0]] = emit_dif(*iters[0])
        for idx, (ci, m) in enumerate(iters):
            dif = difs.pop((ci, m)) if prefetch_dif else emit_dif(ci, m)
            if no_act:
                absd = absd_static
            else:
                absd = absp.tile([128, CHUNK], absd_dt, name="absd")
                nc.scalar.activation(absd[:], dif[:], mybir.ActivationFunctionType.Abs)

            # optionally prefetch next iteration's dif matmuls
            if prefetch_dif and idx + 1 < len(iters):
                difs[iters[idx + 1]] = emit_dif(*iters[idx + 1])

            sm = psum_sum.tile([128, CHUNK], fp32, name="sum_bank")
            w = qT[:, m * 128 : (m + 1) * 128]
            for h in range(CHUNK // MM_N):
                hs = slice(h * MM_N, (h + 1) * MM_N)
                cs = slice(ci * CHUNK + h * MM_N, ci * CHUNK + (h + 1) * MM_N)
                nc.tensor.matmul(sm[:, hs], w, dsumT[:, cs], start=True, stop=True)
            if no_dve:
                continue
            if ci == 0:
                # mask the diagonal pair block (rotated into chunk 0)
                ms = slice(m * 64, m * 64 + 64)
                nc.vector.tensor_tensor(sm[:, ms], sm[:, ms], heye[:, 0:64],
                                        op=mybir.AluOpType.subtract)
            seed = -1e30 if ci == 0 else chain[ci - 1][:, m : m + 1]
            tr2 = trashp.tile([128, CHUNK], fp32, name="fused_trash")
            nc.vector._custom_dve(
                FUSED_OP, out=tr2[:], in0=sm[:], in1=absd[:],
                s1=seed,
                accum_out=chain[ci][:, m : m + 1])

        # fold the exact partner score back in (replaces the masked diag pair)
        negib = small.tile([D, M_TILES], fp32, name="negib_t")
        if not no_dve:
            nc.vector.tensor_tensor(negib[:], chain[N_CHUNKS - 1][:], par_acc[:],
                                    op=mybir.AluOpType.max)
        else:
            nc.vector.memset(negib[:], 0.0)

        loop_cm.close()

        nc.sync.dma_start(out_d.ap()[0], negib[:])
        nc.sync.dma_start(out_d.ap()[1], accs["qd_s"][:])
        nc.sync.dma_start(out_d.ap()[2], accs["qd_d"][:])
        nc.sync.dma_start(out_d.ap()[3], accs["qn_s"][:])
        nc.sync.dma_start(out_d.ap()[4], accs["qn_d"][:])

    nc.compile()
    return nc


def _get_compiled():
    global _COMPILED
    if _COMPILED is None:
        _COMPILED = _build()
    return _COMPILED


def _prep_inputs(q, d, nd):
    q = np.ascontiguousarray(np.asarray(q, dtype=np.float32))
    d = np.ascontiguousarray(np.asarray(d, dtype=np.float32))
    nd = np.ascontiguousarray(np.asarray(nd, dtype=np.float32))

    qT_bf = np.ascontiguousarray(q.T.astype(ml_dtypes.bfloat16))          # [D, B]
    dsum = ((d[0::2] + d[1::2]) * np.float32(0.5))                         # [PC, D]
    ddif = ((d[0::2] - d[1::2]) * np.float32(0.5))
    dsumT = np.ascontiguousarray(dsum.T.astype(ml_dtypes.bfloat16))        # [D, PC]
    ddifT = np.ascontiguousarray(ddif.T.astype(ml_dtypes.bfloat16))
    dsw = d[np.arange(B) ^ 1]                                              # partner rows

    heye = np.zeros((D, 64), dtype=np.float32)
    heye[np.arange(D), np.arange(D) // 2] = BIG

    half = np.float32(0.5)
    dots_full = {
        "qd_s": (q + d) * half, "qd_d": (q - d) * half,
        "qn_s": (q + nd) * half, "qn_d": (q - nd) * half,
        "qw_s": (q + dsw) * half, "qw_d": (q - dsw) * half,
    }

    in_maps = []
    for c in range(NCORES):
        r0 = c * R
        rot = np.roll(np.arange(PC), -(r0 // 2))
        im = {
            "qT": np.ascontiguousarray(qT_bf[:, r0 : r0 + R]),
            "dsumT": np.ascontiguousarray(dsumT[:, rot]),
            "ddifT": np.ascontiguousarray(ddifT[:, rot]),
            "heye": heye,
        }
        for n, arr in dots_full.items():
            im[n] = np.ascontiguousarray(arr[r0 : r0 + R])
        in_maps.append(im)
    return in_maps


def _gather(results):
    negib = np.empty(B, dtype=np.float32)
    pos = np.empty(B, dtype=np.float32)
    neg = np.empty(B, dtype=np.float32)
    for c in range(NCORES):
        o = results[c]["out"]  # [5, 128, M_TILES]
        r0 = c * R
        negib[r0 : r0 + R] = o[0].T.reshape(-1)
        # dot = sum((q+x)^2)/4 - sum((q-x)^2)/4 with the 1/2 prescale
        # already applied on the host: dot = sum(s^2) - sum(d^2)
        pos[r0 : r0 + R] = (o[1] - o[2]).T.reshape(-1)
        neg[r0 : r0 + R] = (o[3] - o[4]).T.reshape(-1)
    return negib, pos, neg


def kernel(query_embeddings, doc_embeddings, neg_doc_embeddings):
    nc = _get_compiled()
    in_maps = _prep_inputs(query_embeddings, doc_embeddings, neg_doc_embeddings)
    res = run_bass_kernel_spmd(nc, in_maps, core_ids=list(range(NCORES)))
    negib, pos, neg = _gather(res.results)

    pos64 = pos.astype(np.float64)
    l1 = np.mean(np.logaddexp(0.0, neg.astype(np.float64) - pos64))
    l2 = np.mean(np.logaddexp(0.0, negib.astype(np.float64) - pos64))
    return np.float32((l1 + l2) / 2.0)



# revision 16
# speedup vs baseline: 1.2151x; 1.2151x over previous
"""BiPairwiseNegativeCELoss Trainium2 kernel (8-core data-parallel).

loss = ( mean(softplus(neg - pos)) + mean(softplus(neg_ib - pos)) ) / 2
  pos    = rowwise dot(q, d)                (diag of q @ d.T)
  neg    = rowwise dot(q, nd)
  neg_ib = rowmax of (q @ d.T - BIG*eye)    (hardest in-batch negative)

Sharding: batch rows split across 8 cores (2048 rows each); every core
streams the full doc matrix as the matmul moving operand.

Every score column must be consumed once from PSUM by a non-PE engine.
Measured per-op DVE cost is strongly non-linear in op size (fused op:
~0.98 ns/elem at 512 elements vs ~1.6 ns/elem at 1024), so the
pipeline uses [128, CHUNK] fp32 PSUM tiles with CHUNK=512: two pools
(dif / sum) of `bufs` slots each, giving deep double-buffering.

Two consumption paths per 128-row m-tile (16384 columns each), chosen
per m-tile plan (P pair-chunks, R raw-chunks; 2P + R = 16384/CHUNK):
 * pair chunks:  max(a,b) = (a+b)/2 + |(a-b)/2|:
     ssum = q @ DsumT, sdif = q @ DdifT           [TensorE]
     |sdif|  PSUM -> SBUF f16                     [ScalarE Abs]
     rowmax(ssum + |sdif|), chained per chunk     [VectorE fused
        custom-DVE op: body=Src0+Src1, accum=maxx, seed=C1]
 * raw chunks, consumed as sum(exp(s - C)) per row [ScalarE Exp+accum]
   log-sum-exp upper-bounds the max; the +log(1+tail) bias (~0.1 on a
   ~45-score max, only for rows whose max lives in the raw slice) is
   far below tolerance.  R balances VectorE vs ScalarE load.

The diagonal pair {s_ii, s_i,i^1} is excluded with a -1e6 "half-eye"
mask on the ssum chunk that contains this m-tile's rotated diagonal
block; the partner score s_i,i^1 is re-added exactly via a rowwise
dot.  Per-core pair-columns are rotated so every core's diagonal
block lands at the same static position (keeps the program SPMD).
Rowwise dots (pos/neg/partner) use the ScalarE square trick:
4*q.x = sum((q+x)^2) - sum((q-x)^2).  Softplus + means run on the
host in float64 on the tiny per-row vectors.
"""

import numpy as np
import ml_dtypes

import concourse.bacc as bacc
import concourse.tile as tile
import concourse.mybir as mybir
import concourse.dve_ops as dve_ops
from concourse.dve_spec import Spec, Src0, Src1, C1, maxx, lower, _has_src1
from concourse.dve_uop import DveOpSpec
from concourse.bass_utils import run_bass_kernel_spmd
from contextlib import ExitStack

B = 16384          # batch
D = 128            # embedding dim
NCORES = 8
R = B // NCORES    # rows per core = 2048
M_TILES = R // 128          # 16 row tiles per core
PC = B // 2                 # pair columns = 8192
CHUNK = 512                 # PSUM elements per consumer op (1 bank)
MM_N = 512                  # moving free dim per matmul
BIG = 1e6
EXPC = 45.0                 # exp bias: sum(exp(s - EXPC))

N_CH = 16384 // CHUNK       # 32 column-chunks per m-tile
PAIR_CH = 16                # pair chunks per m-tile (of CHUNK pairs each)
RAW_CH = N_CH - 2 * PAIR_CH # raw chunks per m-tile

_COMPILED = {}


def _ref_tt_add_maxred(in0, in1, c0, c1, c2):
    P = in0.shape[0]
    body = (in0.astype(np.float32).reshape(P, -1)
            + np.asarray(in1, np.float32).reshape(P, -1))
    return body, dve_ops._accum_ref(body, c1, maxx, False)


def _register_fused_op():
    """out = in0 + in1 ; accum_out = max(rowmax(out), seed[C1])."""
    name = "TT_ADD_MAXREDUCE_ANT"
    if name in dve_ops._SUB_OPCODE_FOR_NAME:
        return next(op for op in dve_ops.OPS if op.name == name)
    op = dve_ops.DveOp(
        name,
        Spec(body=Src0 + Src1, accum=maxx, accum_init=C1,
             reference=_ref_tt_add_maxred),
        subdim=False,
        uops_sha={},
    )
    row = max(dve_ops._SUB_OPCODE_FOR_NAME.values()) + 1
    assert row < 0x20
    dve_ops.OPS.append(op)
    dve_ops.CUSTOM_DVE_SPECS[name] = op.spec
    dve_ops._SUB_OPCODE_FOR_NAME[name] = row
    for ver in ("v3", "v4"):
        spec = DveOpSpec(name=name, opcode=row, uops=lower(op.spec, ver=ver),
                         rd1_en=_has_src1(op.spec))
        op.uops_sha[ver] = spec.sha(ver)
    return op


FUSED_OP = _register_fused_op()


def _build(repeat=1, pair_ch=PAIR_CH, absd_f16=True, bufs=3, m_outer=False,
           dummy_dve=0, dummy_act=0):
    raw_ch = N_CH - 2 * pair_ch
    PP = pair_ch * CHUNK            # pair columns used per m-tile
    RAW = raw_ch * CHUNK            # raw columns
    fp32, bf16 = mybir.dt.float32, mybir.dt.bfloat16
    f16 = mybir.dt.float16
    absd_dt = f16 if absd_f16 else fp32
    nc = bacc.Bacc("TRN2", target_bir_lowering=False, debug=False)

    qT_d = nc.dram_tensor("qT", [D, R], bf16, kind="ExternalInput")
    dsumT_d = nc.dram_tensor("dsumT", [D, PP], bf16, kind="ExternalInput")
    ddifT_d = nc.dram_tensor("ddifT", [D, PP], bf16, kind="ExternalInput")
    if raw_ch:
        rawT_d = nc.dram_tensor("rawT", [D, RAW], bf16, kind="ExternalInput")
    dot_names = ["qd_s", "qd_d", "qn_s", "qn_d", "qw_s", "qw_d"]
    dot_drams = {n: nc.dram_tensor(n, [R, D], fp32, kind="ExternalInput")
                 for n in dot_names}
    heye_d = nc.dram_tensor("heye", [D, 64], fp32, kind="ExternalInput")
    out_d = nc.dram_tensor("out", [5, D, M_TILES], fp32, kind="ExternalOutput")
    if raw_ch:
        expacc_d = nc.dram_tensor("expacc", [D, M_TILES * raw_ch], fp32,
                                  kind="ExternalOutput")

    with tile.TileContext(nc) as tc, ExitStack() as ctx:
        resid = ctx.enter_context(tc.tile_pool(name="resid", bufs=1))
        dots_in = ctx.enter_context(tc.tile_pool(name="dots_in", bufs=3))
        absp = ctx.enter_context(tc.tile_pool(name="absp", bufs=2 * bufs))
        small = ctx.enter_context(tc.tile_pool(name="small", bufs=1))
        trashp = ctx.enter_context(tc.tile_pool(name="trashp", bufs=2))
        etrash = ctx.enter_context(tc.tile_pool(name="etrash", bufs=2))
        psum_dif = ctx.enter_context(tc.tile_pool(name="psum_dif", bufs=bufs, space="PSUM"))
        psum_sum = ctx.enter_context(tc.tile_pool(name="psum_sum", bufs=bufs, space="PSUM"))

        # resident operands
        qT = resid.tile([D, R], bf16, name="qT_t")
        dsumT = resid.tile([D, PP], bf16, name="dsumT_t")
        ddifT = resid.tile([D, PP], bf16, name="ddifT_t")
        if raw_ch:
            rawT = resid.tile([D, RAW], bf16, name="rawT_t")
        heye = resid.tile([D, 64], fp32, name="heye_t")

        nc.sync.dma_start(qT[:], qT_d.ap())
        nc.sync.dma_start(heye[:], heye_d.ap())
        for ci in range(0, PP, 1024):
            sl = slice(ci, min(ci + 1024, PP))
            nc.sync.dma_start(ddifT[:, sl], ddifT_d.ap()[:, sl])
            nc.sync.dma_start(dsumT[:, sl], dsumT_d.ap()[:, sl])
        for rc in range(0, RAW, 1024):
            sl = slice(rc, min(rc + 1024, RAW))
            nc.sync.dma_start(rawT[:, sl], rawT_d.ap()[:, sl])

        # staging for per-row results
        accs = {n: small.tile([D, M_TILES], fp32, name=f"acc_{n}")
                for n in dot_names}
        par_acc = small.tile([D, M_TILES], fp32, name="par_acc")
        # chain[ci] holds the running rowmax after pair-chunk ci
        chain = [small.tile([D, M_TILES], fp32, name=f"chain_{ci}")
                 for ci in range(pair_ch)]
        if raw_ch:
            expacc = small.tile([D, M_TILES * raw_ch], fp32, name="expacc_t")
            expbias = small.tile([D, 1], fp32, name="expbias_t")
            nc.vector.memset(expbias[:], -EXPC)

        # ---- rowwise dots via ScalarE Square+accumulate, early ----
        for m in range(M_TILES):
            rs = slice(m * 128, (m + 1) * 128)
            for n in dot_names:
                xt = dots_in.tile([128, D], fp32, name=f"dot_{n}")
                nc.sync.dma_start(xt[:], dot_drams[n].ap()[rs, :])
                tr = trashp.tile([128, D], f16, name="dot_trash", tag="trash")
                nc.scalar.activation(tr[:], xt[:],
                                     mybir.ActivationFunctionType.Square,
                                     accum_out=accs[n][:, m : m + 1])
        # partner seed: host pre-scales (q±w) by 1/2, so the squared-sum
        # difference is exactly q.w
        nc.vector.tensor_tensor(par_acc[:], accs["qw_s"][:], accs["qw_d"][:],
                                op=mybir.AluOpType.subtract)

        # ---- main pipeline ----
        loop_cm = ExitStack()
        if repeat > 1:
            loop_cm.enter_context(tc.For_i(
                0, repeat, 1,
                hint_engines=(mybir.EngineType.PE, mybir.EngineType.DVE,
                              mybir.EngineType.Activation)))
        if m_outer:
            pair_iters = [("p", ci, m) for m in range(M_TILES)
                          for ci in range(pair_ch)]
        else:
            pair_iters = [("p", ci, m) for ci in range(pair_ch)
                          for m in range(M_TILES)]
        raw_iters = [("r", rc, m) for rc in range(raw_ch)
                     for m in range(M_TILES)]
        # interleave raw iters evenly among pair iters
        iters, ia, ib = [], 0, 0
        while ia < len(pair_iters) or ib < len(raw_iters):
            if ib >= len(raw_iters) or (
                    ia < len(pair_iters)
                    and ia * len(raw_iters) <= ib * len(pair_iters)):
                iters.append(pair_iters[ia]); ia += 1
            else:
                iters.append(raw_iters[ib]); ib += 1

        if dummy_dve or dummy_act:
            dsrc = resid.tile([D, CHUNK], mybir.dt.float32, name="dsrc")
            nc.vector.memset(dsrc[:], 1.0)
            dabs = resid.tile([D, CHUNK], absd_dt, name="dabs")
            nc.vector.memset(dabs[:], 0.5)
            dtrash = resid.tile([D, CHUNK], mybir.dt.float16, name="dtrash")
            dacc = resid.tile([D, 1], mybir.dt.float32, name="dacc")

        for kind, ci, m in iters:
            w = qT[:, m * 128 : (m + 1) * 128]
            for _ in range(dummy_dve):
                nc.vector._custom_dve(
                    FUSED_OP, out=dtrash[:], in0=dsrc[:], in1=dabs[:],
                    s1=-1e30, accum_out=dacc[:])
            for _ in range(dummy_act):
                nc.scalar.activation(dtrash[:], dsrc[:],
                                     mybir.ActivationFunctionType.Abs)
            if kind == "r":
                sc = psum_dif.tile([128, CHUNK], fp32, name="raw_bank", tag="bank")
                cs = slice(ci * CHUNK, (ci + 1) * CHUNK)
                nc.tensor.matmul(sc[:], w, rawT[:, cs], start=True, stop=True)
                et = etrash.tile([128, CHUNK], f16, name="exp_trash")
                slot = m * raw_ch + ci
                nc.scalar.activation(et[:], sc[:],
                                     mybir.ActivationFunctionType.Exp,
                                     bias=expbias[:],
                                     accum_out=expacc[:, slot : slot + 1])
                continue

            dif = psum_dif.tile([128, CHUNK], fp32, name="dif_bank", tag="bank")
            cs = slice(ci * CHUNK, (ci + 1) * CHUNK)
            nc.tensor.matmul(dif[:], w, ddifT[:, cs], start=True, stop=True)
            absd = absp.tile([128, CHUNK], absd_dt, name="absd")
            nc.scalar.activation(absd[:], dif[:], mybir.ActivationFunctionType.Abs)

            sm = psum_sum.tile([128, CHUNK], fp32, name="sum_bank")
            nc.tensor.matmul(sm[:], w, dsumT[:, cs], start=True, stop=True)
            if ci == (m * 64) // CHUNK:
                # mask this m-tile's diagonal pair block
                ms = slice((m * 64) % CHUNK, (m * 64) % CHUNK + 64)
                nc.vector.tensor_tensor(sm[:, ms], sm[:, ms], heye[:, 0:64],
                                        op=mybir.AluOpType.subtract)
            seed = -1e30 if ci == 0 else chain[ci - 1][:, m : m + 1]
            tr2 = trashp.tile([128, CHUNK], f16, name="fused_trash", tag="trash")
            nc.vector._custom_dve(
                FUSED_OP, out=tr2[:], in0=sm[:], in1=absd[:],
                s1=seed,
                accum_out=chain[ci][:, m : m + 1])

        # fold the exact partner score back in (replaces the masked diag pair)
        negib = small.tile([D, M_TILES], fp32, name="negib_t")
        nc.vector.tensor_tensor(negib[:], chain[pair_ch - 1][:], par_acc[:],
                                op=mybir.AluOpType.max)

        loop_cm.close()

        nc.sync.dma_start(out_d.ap()[0], negib[:])
        nc.sync.dma_start(out_d.ap()[1], accs["qd_s"][:])
        nc.sync.dma_start(out_d.ap()[2], accs["qd_d"][:])
        nc.sync.dma_start(out_d.ap()[3], accs["qn_s"][:])
        nc.sync.dma_start(out_d.ap()[4], accs["qn_d"][:])
        if raw_ch:
            nc.sync.dma_start(expacc_d.ap(), expacc[:])

    nc.compile()
    return nc


def _get_compiled(pair_ch=PAIR_CH):
    if pair_ch not in _COMPILED:
        _COMPILED[pair_ch] = _build(pair_ch=pair_ch)
    return _COMPILED[pair_ch]


def _prep_inputs(q, d, nd, pair_ch=PAIR_CH):
    PP = pair_ch * CHUNK
    q = np.ascontiguousarray(np.asarray(q, dtype=np.float32))
    d = np.ascontiguousarray(np.asarray(d, dtype=np.float32))
    nd = np.ascontiguousarray(np.asarray(nd, dtype=np.float32))

    qT_bf = np.ascontiguousarray(q.T.astype(ml_dtypes.bfloat16))          # [D, B]
    dT_bf = np.ascontiguousarray(d.T.astype(ml_dtypes.bfloat16))          # [D, B]
    dsum = ((d[0::2] + d[1::2]) * np.float32(0.5))                         # [PC, D]
    ddif = ((d[0::2] - d[1::2]) * np.float32(0.5))
    dsumT = np.ascontiguousarray(dsum.T.astype(ml_dtypes.bfloat16))        # [D, PC]
    ddifT = np.ascontiguousarray(ddif.T.astype(ml_dtypes.bfloat16))
    dsw = d[np.arange(B) ^ 1]                                              # partner rows

    heye = np.zeros((D, 64), dtype=np.float32)
    heye[np.arange(D), np.arange(D) // 2] = BIG

    half = np.float32(0.5)
    dots_full = {
        "qd_s": (q + d) * half, "qd_d": (q - d) * half,
        "qn_s": (q + nd) * half, "qn_d": (q - nd) * half,
        "qw_s": (q + dsw) * half, "qw_d": (q - dsw) * half,
    }

    in_maps = []
    for c in range(NCORES):
        r0 = c * R
        rot = np.roll(np.arange(PC), -(r0 // 2))
        im = {
            "qT": np.ascontiguousarray(qT_bf[:, r0 : r0 + R]),
            "dsumT": np.ascontiguousarray(dsumT[:, rot[:PP]]),
            "ddifT": np.ascontiguousarray(ddifT[:, rot[:PP]]),
            "heye": heye,
        }
        if PP < PC:
            rp = rot[PP:]
            idx = np.empty(2 * len(rp), dtype=np.int64)
            idx[0::2] = 2 * rp
            idx[1::2] = 2 * rp + 1
            im["rawT"] = np.ascontiguousarray(dT_bf[:, idx])
        for n, arr in dots_full.items():
            im[n] = np.ascontiguousarray(arr[r0 : r0 + R])
        in_maps.append(im)
    return in_maps


def _gather(results, pair_ch=PAIR_CH):
    raw_ch = N_CH - 2 * pair_ch
    negib = np.empty(B, dtype=np.float32)
    pos = np.empty(B, dtype=np.float32)
    neg = np.empty(B, dtype=np.float32)
    for c in range(NCORES):
        o = results[c]["out"]  # [5, 128, M_TILES]
        r0 = c * R
        nib = o[0].T.reshape(-1)
        if raw_ch:
            ea = results[c]["expacc"]  # [128, M_TILES * raw_ch]
            es = ea.reshape(128, M_TILES, raw_ch).sum(axis=2)  # [128, M]
            lse = np.log(np.maximum(es.T.reshape(-1), 1e-38)) + EXPC
            nib = np.maximum(nib, lse.astype(np.float32))
        negib[r0 : r0 + R] = nib
        # dot = sum((q+x)^2)/4 - sum((q-x)^2)/4 with the 1/2 prescale
        # already applied on the host: dot = sum(s^2) - sum(d^2)
        pos[r0 : r0 + R] = (o[1] - o[2]).T.reshape(-1)
        neg[r0 : r0 + R] = (o[3] - o[4]).T.reshape(-1)
    return negib, pos, neg


def kernel(query_embeddings, doc_embeddings, neg_doc_embeddings):
    nc = _get_compiled()
    in_maps = _prep_inputs(query_embeddings, doc_embeddings, neg_doc_embeddings)
    res = run_bass_kernel_spmd(nc, in_maps, core_ids=list(range(NCORES)))
    negib, pos, neg = _gather(res.results)

    pos64 = pos.astype(np.float64)
    l1 = np.mean(np.logaddexp(0.0, neg.astype(np.float64) - pos64))
    l2 = np.mean(np.logaddexp(0.0, negib.astype(np.float64) - pos64))
    return np.float32((l1 + l2) / 2.0)


# revision 17
# speedup vs baseline: 1.4088x; 1.1594x over previous
"""BiPairwiseNegativeCELoss Trainium2 kernel (8-core data-parallel).

loss = ( mean(softplus(neg - pos)) + mean(softplus(neg_ib - pos)) ) / 2
  pos    = rowwise dot(q, d)                (diag of q @ d.T)
  neg    = rowwise dot(q, nd)
  neg_ib = rowmax of (q @ d.T - BIG*eye)    (hardest in-batch negative)

Sharding: batch rows split across 8 cores (2048 rows each); every core
streams the full doc matrix as the matmul moving operand.

The row-max over 16384 columns is the bottleneck (PSUM can only be read
by the Vector/Scalar engines at ~1 elem/lane/cycle). We halve it with a
pair-max decomposition:

  max(a, b) = (a+b)/2 + |(a-b)/2|

  ssum = q @ DsumT  (Dsum = (d_even + d_odd)/2)      [TensorE]
  sdif = q @ DdifT  (Ddif = (d_even - d_odd)/2)      [TensorE]
  |sdif|           PSUM -> SBUF                       [ScalarE Abs]
  rowmax(ssum + |sdif|), seeded/chained per chunk     [VectorE, one
      fused custom-DVE op: body=Src0+Src1, accum=maxx, seed=C1]

The fused op is registered at import time into concourse's custom-DVE
table mechanism (uop table ships inside the NEFF).

The diagonal pair {s_ii, s_i,i^1} is excluded with a -1e6 "half-eye"
mask on the ssum bank; the partner score s_i,i^1 is re-added exactly as
the chunk-0 seed (computed as a rowwise dot). Per-core pair-columns are
rotated so every core's diagonal block lands in chunk 0 at the same
static position (keeps the program SPMD).
Softplus + means run on the host in float64 on the tiny per-row vectors.

(Alternatives explored and measured slower under same-window A/B on
this device: 512-element chunks with deeper PSUM buffering, 2048-chunk
ping-pong, m-outer weight-sharing order, and offloading a column slice
to ScalarE as sum(exp(s-C)) [log-sum-exp upper bound]. The brokered
TRN2 shows up to ~1.8x run-to-run contention drift; only interleaved
same-window comparisons are trustworthy.)
"""

import numpy as np
import ml_dtypes

import concourse.bacc as bacc
import concourse.tile as tile
import concourse.mybir as mybir
import concourse.dve_ops as dve_ops
from concourse.dve_spec import Spec, Src0, Src1, C1, maxx, lower, _has_src1
from concourse.dve_uop import DveOpSpec
from concourse.bass_utils import run_bass_kernel_spmd
from contextlib import ExitStack

B = 16384          # batch
D = 128            # embedding dim
NCORES = 8
R = B // NCORES    # rows per core = 2048
M_TILES = R // 128          # 16 row tiles per core
PC = B // 2                 # pair columns = 8192
CHUNK = 1024                # pair columns per pipeline iteration
N_CHUNKS = PC // CHUNK      # 8
MM_N = 512                  # moving free dim per matmul
BIG = 1e6

_COMPILED = None


def _ref_tt_add_maxred(in0, in1, c0, c1, c2):
    P = in0.shape[0]
    body = (in0.astype(np.float32).reshape(P, -1)
            + np.asarray(in1, np.float32).reshape(P, -1))
    return body, dve_ops._accum_ref(body, c1, maxx, False)


def _register_fused_op():
    """out = in0 + in1 ; accum_out = max(rowmax(out), seed[C1])."""
    name = "TT_ADD_MAXREDUCE_ANT"
    if name in dve_ops._SUB_OPCODE_FOR_NAME:
        return next(op for op in dve_ops.OPS if op.name == name)
    op = dve_ops.DveOp(
        name,
        Spec(body=Src0 + Src1, accum=maxx, accum_init=C1,
             reference=_ref_tt_add_maxred),
        subdim=False,
        uops_sha={},
    )
    row = max(dve_ops._SUB_OPCODE_FOR_NAME.values()) + 1
    assert row < 0x20
    dve_ops.OPS.append(op)
    dve_ops.CUSTOM_DVE_SPECS[name] = op.spec
    dve_ops._SUB_OPCODE_FOR_NAME[name] = row
    for ver in ("v3", "v4"):
        spec = DveOpSpec(name=name, opcode=row, uops=lower(op.spec, ver=ver),
                         rd1_en=_has_src1(op.spec))
        op.uops_sha[ver] = spec.sha(ver)
    return op


FUSED_OP = _register_fused_op()


def _build(repeat=1, absd_f16=True):
    fp32, bf16 = mybir.dt.float32, mybir.dt.bfloat16
    absd_dt = mybir.dt.float16 if absd_f16 else fp32
    nc = bacc.Bacc("TRN2", target_bir_lowering=False, debug=False)

    qT_d = nc.dram_tensor("qT", [D, R], bf16, kind="ExternalInput")
    dsumT_d = nc.dram_tensor("dsumT", [D, PC], bf16, kind="ExternalInput")
    ddifT_d = nc.dram_tensor("ddifT", [D, PC], bf16, kind="ExternalInput")
    # (q±x) shards for the rowwise dots via the square trick:
    # 4*q.x = sum((q+x)^2) - sum((q-x)^2), accumulated on ScalarE
    dot_names = ["qd_s", "qd_d", "qn_s", "qn_d", "qw_s", "qw_d"]
    dot_drams = {n: nc.dram_tensor(n, [R, D], fp32, kind="ExternalInput")
                 for n in dot_names}
    heye_d = nc.dram_tensor("heye", [D, 64], fp32, kind="ExternalInput")
    out_d = nc.dram_tensor("out", [5, D, M_TILES], fp32, kind="ExternalOutput")

    with tile.TileContext(nc) as tc, ExitStack() as ctx:
        resid = ctx.enter_context(tc.tile_pool(name="resid", bufs=1))
        dots_in = ctx.enter_context(tc.tile_pool(name="dots_in", bufs=3))
        absp = ctx.enter_context(tc.tile_pool(name="absp", bufs=4))
        small = ctx.enter_context(tc.tile_pool(name="small", bufs=1))
        trashp = ctx.enter_context(tc.tile_pool(name="trashp", bufs=2))
        psum_dif = ctx.enter_context(tc.tile_pool(name="psum_dif", bufs=2, space="PSUM"))
        psum_sum = ctx.enter_context(tc.tile_pool(name="psum_sum", bufs=2, space="PSUM"))

        # resident operands
        qT = resid.tile([D, R], bf16, name="qT_t")
        dsumT = resid.tile([D, PC], bf16, name="dsumT_t")
        ddifT = resid.tile([D, PC], bf16, name="ddifT_t")
        heye = resid.tile([D, 64], fp32, name="heye_t")

        nc.sync.dma_start(qT[:], qT_d.ap())
        nc.sync.dma_start(heye[:], heye_d.ap())
        for ci in range(N_CHUNKS):
            sl = slice(ci * CHUNK, (ci + 1) * CHUNK)
            nc.sync.dma_start(ddifT[:, sl], ddifT_d.ap()[:, sl])
            nc.sync.dma_start(dsumT[:, sl], dsumT_d.ap()[:, sl])

        # staging for per-row results
        accs = {n: small.tile([D, M_TILES], fp32, name=f"acc_{n}")
                for n in dot_names}
        par_acc = small.tile([D, M_TILES], fp32, name="par_acc")
        # chain[ci] holds the running rowmax after chunk ci (per m-tile col)
        chain = [small.tile([D, M_TILES], fp32, name=f"chain_{ci}")
                 for ci in range(N_CHUNKS)]

        # ---- rowwise dots via ScalarE Square+accumulate, early ----
        f16 = mybir.dt.float16
        for m in range(M_TILES):
            rs = slice(m * 128, (m + 1) * 128)
            for n in dot_names:
                xt = dots_in.tile([128, D], fp32, name=f"dot_{n}")
                nc.sync.dma_start(xt[:], dot_drams[n].ap()[rs, :])
                tr = trashp.tile([128, D], f16, name="dot_trash")
                nc.scalar.activation(tr[:], xt[:],
                                     mybir.ActivationFunctionType.Square,
                                     accum_out=accs[n][:, m : m + 1])
        # partner seed: host pre-scales (q±w) by 1/2, so the squared-sum
        # difference is exactly q.w
        nc.vector.tensor_tensor(par_acc[:], accs["qw_s"][:], accs["qw_d"][:],
                                op=mybir.AluOpType.subtract)

        # ---- pair-max pipeline ----
        loop_cm = ExitStack()
        if repeat > 1:
            loop_cm.enter_context(tc.For_i(
                0, repeat, 1,
                hint_engines=(mybir.EngineType.PE, mybir.EngineType.DVE,
                              mybir.EngineType.Activation)))
        iters = [(ci, m) for ci in range(N_CHUNKS) for m in range(M_TILES)]

        for ci, m in iters:
            dif = psum_dif.tile([128, CHUNK], fp32, name="dif_bank")
            w = qT[:, m * 128 : (m + 1) * 128]
            for h in range(CHUNK // MM_N):
                cs = slice(ci * CHUNK + h * MM_N, ci * CHUNK + (h + 1) * MM_N)
                nc.tensor.matmul(dif[:, h * MM_N : (h + 1) * MM_N], w,
                                 ddifT[:, cs], start=True, stop=True)
            absd = absp.tile([128, CHUNK], absd_dt, name="absd")
            nc.scalar.activation(absd[:], dif[:], mybir.ActivationFunctionType.Abs)

            sm = psum_sum.tile([128, CHUNK], fp32, name="sum_bank")
            for h in range(CHUNK // MM_N):
                hs = slice(h * MM_N, (h + 1) * MM_N)
                cs = slice(ci * CHUNK + h * MM_N, ci * CHUNK + (h + 1) * MM_N)
                nc.tensor.matmul(sm[:, hs], w, dsumT[:, cs], start=True, stop=True)
            if ci == 0:
                # mask the diagonal pair block (rotated into chunk 0)
                ms = slice(m * 64, m * 64 + 64)
                nc.vector.tensor_tensor(sm[:, ms], sm[:, ms], heye[:, 0:64],
                                        op=mybir.AluOpType.subtract)
            seed = -1e30 if ci == 0 else chain[ci - 1][:, m : m + 1]
            tr2 = trashp.tile([128, CHUNK], fp32, name="fused_trash")
            nc.vector._custom_dve(
                FUSED_OP, out=tr2[:], in0=sm[:], in1=absd[:],
                s1=seed,
                accum_out=chain[ci][:, m : m + 1])

        # fold the exact partner score back in (replaces the masked diag pair)
        negib = small.tile([D, M_TILES], fp32, name="negib_t")
        nc.vector.tensor_tensor(negib[:], chain[N_CHUNKS - 1][:], par_acc[:],
                                op=mybir.AluOpType.max)

        loop_cm.close()

        nc.sync.dma_start(out_d.ap()[0], negib[:])
        nc.sync.dma_start(out_d.ap()[1], accs["qd_s"][:])
        nc.sync.dma_start(out_d.ap()[2], accs["qd_d"][:])
        nc.sync.dma_start(out_d.ap()[3], accs["qn_s"][:])
        nc.sync.dma_start(out_d.ap()[4], accs["qn_d"][:])

    nc.compile()
    return nc


def _get_compiled():
    global _COMPILED
    if _COMPILED is None:
        _COMPILED = _build()
    return _COMPILED


def _prep_inputs(q, d, nd):
    q = np.ascontiguousarray(np.asarray(q, dtype=np.float32))
    d = np.ascontiguousarray(np.asarray(d, dtype=np.float32))
    nd = np.ascontiguousarray(np.asarray(nd, dtype=np.float32))

    qT_bf = np.ascontiguousarray(q.T.astype(ml_dtypes.bfloat16))          # [D, B]
    dsum = ((d[0::2] + d[1::2]) * np.float32(0.5))                         # [PC, D]
    ddif = ((d[0::2] - d[1::2]) * np.float32(0.5))
    dsumT = np.ascontiguousarray(dsum.T.astype(ml_dtypes.bfloat16))        # [D, PC]
    ddifT = np.ascontiguousarray(ddif.T.astype(ml_dtypes.bfloat16))
    dsw = d[np.arange(B) ^ 1]                                              # partner rows

    heye = np.zeros((D, 64), dtype=np.float32)
    heye[np.arange(D), np.arange(D) // 2] = BIG

    half = np.float32(0.5)
    dots_full = {
        "qd_s": (q + d) * half, "qd_d": (q - d) * half,
        "qn_s": (q + nd) * half, "qn_d": (q - nd) * half,
        "qw_s": (q + dsw) * half, "qw_d": (q - dsw) * half,
    }

    in_maps = []
    for c in range(NCORES):
        r0 = c * R
        rot = np.roll(np.arange(PC), -(r0 // 2))
        im = {
            "qT": np.ascontiguousarray(qT_bf[:, r0 : r0 + R]),
            "dsumT": np.ascontiguousarray(dsumT[:, rot]),
            "ddifT": np.ascontiguousarray(ddifT[:, rot]),
            "heye": heye,
        }
        for n, arr in dots_full.items():
            im[n] = np.ascontiguousarray(arr[r0 : r0 + R])
        in_maps.append(im)
    return in_maps


def _gather(results):
    negib = np.empty(B, dtype=np.float32)
    pos = np.empty(B, dtype=np.float32)
    neg = np.empty(B, dtype=np.float32)
    for c in range(NCORES):
        o = results[c]["out"]  # [5, 128, M_TILES]
        r0 = c * R
        negib[r0 : r0 + R] = o[0].T.reshape(-1)
        # dot = sum((q+x)^2)/4 - sum((q-x)^2)/4 with the 1/2 prescale
        # already applied on the host: dot = sum(s^2) - sum(d^2)
        pos[r0 : r0 + R] = (o[1] - o[2]).T.reshape(-1)
        neg[r0 : r0 + R] = (o[3] - o[4]).T.reshape(-1)
    return negib, pos, neg


def kernel(query_embeddings, doc_embeddings, neg_doc_embeddings):
    nc = _get_compiled()
    in_maps = _prep_inputs(query_embeddings, doc_embeddings, neg_doc_embeddings)
    res = run_bass_kernel_spmd(nc, in_maps, core_ids=list(range(NCORES)))
    negib, pos, neg = _gather(res.results)

    pos64 = pos.astype(np.float64)
    l1 = np.mean(np.logaddexp(0.0, neg.astype(np.float64) - pos64))
    l2 = np.mean(np.logaddexp(0.0, negib.astype(np.float64) - pos64))
    return np.float32((l1 + l2) / 2.0)
